# revision 9
# baseline (speedup 1.0000x reference)
"""Trainium2 Bass kernel for nn_AttnNet (BiLSTM + SoftDotAttention + head + BatchNorm).

Strategy (8 NeuronCores, direction-split data parallel):
  - Pair p = (core p, core p+4) jointly owns 16 of the 64 sequences.
    Core p runs the FORWARD LSTM for those 16 seqs; core p+4 runs the
    BACKWARD LSTM (fed time-reversed embeddings, so the on-device
    recurrence is identical SPMD code).
  - Embedding gather happens on host (memory-bound; inputs are shipped
    pre-gathered and pre-transposed per core).
  - Phase 1 (device): xpre = [W_ih^T; bias]^T @ [x; 1]  for all t,b.
  - Phase 2 (device): 256 sequential LSTM steps; per step a
    [512x2048]^T weight-stationary matmul over the 16-seq batch,
    gate nonlinearities on ACT, state update on DVE.
  - AllGather exchanges the 8 "for the pair" hidden histories; each
    core then holds ctx = [h_local, h_remote] for its 8 attended seqs.
    Time-reversal of the remote half is a negative-stride AP read and
    is mathematically safe: attention + mean over T is permutation
    invariant in t, and W_in/W_oa are half-swapped on host for the
    backward cores.
  - Phase 3 (device): SoftDotAttention per seq in fp16 (tensor engine),
    output head in fp32, tanh; y^T [256, 8] out per core.
  - BatchNorm (training-mode batch stats over the full 64-batch) on host.

All matmuls run in float16 (full PE rate, 8x finer mantissa than bf16 --
the BatchNorm at the end amplifies batch-varying errors ~30x, bf16 fails).
"""

import os
import numpy as np
import ml_dtypes  # noqa: F401  (float16 used; bfloat16 available)

import concourse.bass as bass
import concourse.tile as tile
from concourse import bacc
from concourse import mybir
from concourse.bass_utils import run_bass_kernel_spmd
from concourse.masks import make_identity

F16 = mybir.dt.float16
F32 = mybir.dt.float32
U32 = mybir.dt.uint32
AF = mybir.ActivationFunctionType
ALU = mybir.AluOpType

B, E, H, OUT = 64, 300, 512, 256
D = 2 * H
NCORES = 8
BL = 16        # sequences per pair (local batch in the LSTM)
BA = 8         # sequences attended per core
HT = H // 128  # 4 h-tiles
G = 4 * H      # 2048 gate rows
GT = G // 128  # 16 gate tiles
EP = 384       # padded E + bias row (3 k-tiles)
KT1 = EP // 128
DTL = D // 128  # 8 d-tiles
EPS = 1e-5

# Gate-tile order on device: (g, i, f, o) so tanh is one slice and
# sigmoid is one contiguous slice. PyTorch order in weights is (i,f,g,o).
# gate block k (of 4) of the device layout = PT block PERM[k].
GATE_PERM = [2, 0, 1, 3]  # device (g,i,f,o) <- pytorch (i,f,g,o)


def _rev_t(ap, tdim_idx):
    """Return ap with free dim `tdim_idx` (index into ap.ap, incl partition
    dim at 0) reversed (negative stride)."""
    dims = [list(d) for d in ap.ap]
    stride, size = dims[tdim_idx]
    new_off = ap.offset + (size - 1) * stride
    dims[tdim_idx] = [-stride, size]
    return bass.AP(tensor=ap.tensor, offset=new_off, ap=dims)


def build(T=256, use_psum_prefill=True):
    nc = bacc.Bacc('TRN2', target_bir_lowering=False, debug=False, num_devices=NCORES)
    NTOK = T * BL
    TQ = T // 128  # q/k tiles in attention

    xT = nc.declare_dram_parameter("xT", [EP, NTOK], F16, False)
    wihT = nc.declare_dram_parameter("wihT", [EP, G], F16, False)
    whhT = nc.declare_dram_parameter("whhT", [H, G], F16, False)
    winT = nc.declare_dram_parameter("winT", [D, D], F16, False)
    woaT = nc.declare_dram_parameter("woaT", [2 * D, D], F16, False)
    woutT = nc.declare_dram_parameter("woutT", [D, OUT], F32, False)
    bout = nc.declare_dram_parameter("bout", [OUT, 1], F32, False)
    peer = nc.declare_dram_parameter("peerblk", [1, 1], U32, False)
    yT = nc.declare_dram_parameter("yT", [OUT, BA], F32, True)

    xpre_d = nc.dram_tensor("xpre_d", [GT, 128, T, BL], F16)
    bnc_in = nc.dram_tensor("bnc_in", [128, BA, T * HT], F16)
    bnc_out = nc.dram_tensor("bnc_out", [NCORES, 128, BA, T * HT], F16,
                             addr_space="Shared")

    with tile.TileContext(nc) as tc:
        singles = tc.alloc_tile_pool(name="singles", bufs=1)

        # ---- resident weights -------------------------------------------------
        wihT_sb = singles.tile([128, KT1, GT, 128], F16)
        nc.gpsimd.dma_start(out=wihT_sb,
                          in_=wihT.rearrange("(kt p) (gt c) -> p kt gt c",
                                             p=128, c=128))
        whhT_sb = singles.tile([128, HT, GT, 128], F16)
        nc.gpsimd.dma_start(out=whhT_sb,
                          in_=whhT.rearrange("(ht p) (gt c) -> p ht gt c",
                                             p=128, c=128))
        winT_sb = singles.tile([128, DTL, DTL, 128], F16)
        nc.gpsimd.dma_start(out=winT_sb,
                          in_=winT.rearrange("(kt p) (mt c) -> p kt mt c",
                                             p=128, c=128))
        woaT_sb = singles.tile([128, 2 * DTL, DTL, 128], F16)
        nc.gpsimd.dma_start(out=woaT_sb,
                          in_=woaT.rearrange("(kt p) (mt c) -> p kt mt c",
                                             p=128, c=128))
        woutT_sb = singles.tile([128, DTL, 2, 128], F32)
        nc.gpsimd.dma_start(out=woutT_sb,
                          in_=woutT.rearrange("(kt p) (mt c) -> p kt mt c",
                                              p=128, c=128))
        bout_sb = singles.tile([128, 2], F32)
        nc.gpsimd.dma_start(out=bout_sb,
                          in_=bout.rearrange("(mt p) one -> p (mt one)", p=128))
        ident = singles.tile([128, 128], F16)
        make_identity(nc, ident)

        h_hist = singles.tile([128, BL, T + 1, HT], F16)
        nc.vector.memset(h_hist[:, :, 0, :], 0.0)
        c_tiles = [singles.tile([128, HT, BL], F32, name=f"c{i}") for i in range(2)]
        nc.vector.memset(c_tiles[0], 0.0)

        # ---- phase 1: xpre ----------------------------------------------------
        xT_v = xT.rearrange("(kt p) n -> p kt n", p=128)
        NSL = NTOK // 512
        with tc.tile_pool(name="p1x", bufs=3) as p1x, \
             tc.tile_pool(name="p1ps", bufs=2, space="PSUM") as p1ps, \
             tc.tile_pool(name="p1st", bufs=4) as p1st:
            for s in range(NSL):
                xt_t = p1x.tile([128, KT1, 512], F16)
                nc.gpsimd.dma_start(out=xt_t, in_=xT_v[:, :, s * 512:(s + 1) * 512])
                for gt in range(GT):
                    ps = p1ps.tile([128, 512], F32)
                    for kt in range(KT1):
                        nc.tensor.matmul(ps, lhsT=wihT_sb[:, kt, gt, :],
                                         rhs=xt_t[:, kt, :],
                                         start=(kt == 0), stop=(kt == KT1 - 1))
                    st = p1st.tile([128, 512], F16)
                    if gt % 2 == 0:
                        nc.scalar.copy(st, ps)
                    else:
                        nc.vector.tensor_copy(st, ps)
                    nc.gpsimd.dma_start(
                        out=xpre_d[gt, :, s * (512 // BL):(s + 1) * (512 // BL), :],
                        in_=st.rearrange("p (t b) -> p t b", b=BL))

        # ---- phase 2: recurrence ---------------------------------------------
        xpre_v = xpre_d.rearrange("gt p t b -> p gt t b")
        CH = 32  # t-chunk per xpre DMA
        with tc.tile_pool(name="p2x", bufs=2) as p2x, \
             tc.tile_pool(name="p2ps", bufs=2, space="PSUM") as p2ps, \
             tc.tile_pool(name="p2g", bufs=3) as p2g, \
             tc.tile_pool(name="p2s", bufs=6) as p2s:
            xpc = None
            for t in range(T):
                tl = t % CH
                if tl == 0:
                    xpc = p2x.tile([128, GT, CH, BL], F16)
                    nc.gpsimd.dma_start(out=xpc, in_=xpre_v[:, :, t:t + CH, :])
                ps2 = p2ps.tile([128, GT, BL], F32)
                if use_psum_prefill:
                    nc.vector.tensor_copy(ps2, xpc[:, :, tl, :])
                rhs_t = [h_hist[:, :, t, ht] for ht in range(HT)]
                for ht in range(HT):
                    for gt in range(GT):
                        nc.tensor.matmul(
                            ps2[:, gt, :], lhsT=whhT_sb[:, ht, gt, :],
                            rhs=rhs_t[ht],
                            start=(ht == 0 and not use_psum_prefill),
                            stop=(ht == HT - 1), skip_group_check=True)
                if use_psum_prefill:
                    act = p2g.tile([128, GT, BL], F32)
                    gsrc = ps2
                else:
                    gsb = p2g.tile([128, GT, BL], F32)
                    nc.vector.scalar_tensor_tensor(
                        gsb, in0=ps2, scalar=1.0, in1=xpc[:, :, tl, :],
                        op0=ALU.mult, op1=ALU.add)
                    act = p2g.tile([128, GT, BL], F32, name="act")
                    gsrc = gsb
                # gate order (g, i, f, o): tanh on [0:4], sigmoid on [4:16]
                nc.scalar.activation(act[:, 0:4, :], gsrc[:, 0:4, :], AF.Tanh)
                nc.scalar.activation(act[:, 4:16, :], gsrc[:, 4:16, :], AF.Sigmoid)
                c_prev, c_new = c_tiles[t % 2], c_tiles[(t + 1) % 2]
                ig = p2s.tile([128, HT, BL], F32)
                nc.vector.tensor_tensor(ig, act[:, 4:8, :], act[:, 0:4, :], ALU.mult)
                fc = p2s.tile([128, HT, BL], F32)
                nc.vector.tensor_tensor(fc, act[:, 8:12, :], c_prev, ALU.mult)
                nc.vector.tensor_tensor(c_new, ig, fc, ALU.add)
                thc = p2s.tile([128, HT, BL], F32)
                nc.scalar.activation(thc, c_new, AF.Tanh)
                h_out = h_hist[:, :, t + 1, :].rearrange("p b ht -> p ht b")
                nc.vector.tensor_tensor(h_out, act[:, 12:16, :], thc, ALU.mult)

        # ---- exchange ---------------------------------------------------------
        nc.gpsimd.dma_start(
            out=bnc_in.rearrange("p b (t h) -> p b t h", h=HT),
            in_=h_hist[:, BA:BL, 1:T + 1, :])
        nc.gpsimd.collective_compute(
            "AllGather", ALU.bypass,
            ins=[bnc_in[:]], outs=[bnc_out[:]],
            replica_groups=[list(range(NCORES))])

        h_rem = singles.tile([128, BA, T, HT], F16)
        peer_sb = singles.tile([1, 1], U32)
        nc.gpsimd.dma_start(out=peer_sb, in_=peer[:, :])
        reg = nc.gpsimd.alloc_register("peerblk_reg")
        nc.gpsimd.reg_load(reg, peer_sb[0:1, 0:1])
        sv = nc.gpsimd.snap(reg, donate=True, min_val=0, max_val=NCORES - 1)
        nc.gpsimd.dma_start(
            out=h_rem,
            in_=bnc_out.rearrange("r p b (t h) -> r p b t h", h=HT)[
                bass.ds(sv, 1), :, :, :, :].rearrange(
                    "o p b t h -> p (o b) t h"))
        # time-reverse the peer block in SBUF (negative-stride DVE read):
        # permutation-safe under attention+mean, and makes local/remote halves
        # of each ctx token refer to the same true timestep.
        h_rev = singles.tile([128, BA, T, HT], F16)
        nc.vector.tensor_copy(h_rev, _rev_t(h_rem, 2))

        def ctxT(b, dt):
            if dt < HT:
                return h_hist[:, b, 1:T + 1, dt]
            return h_rev[:, b, :, dt - HT]

        # ---- phase 3: attention ----------------------------------------------
        ctxo = singles.tile([128, DTL, BA], F32)
        with tc.tile_pool(name="p3ps", bufs=3, space="PSUM") as p3ps, \
             tc.tile_pool(name="p3tr", bufs=2, space="PSUM") as p3tr, \
             tc.tile_pool(name="p3a", bufs=2) as p3a, \
             tc.tile_pool(name="p3b", bufs=2) as p3b:
            for b in range(BA):
                # ctx in [k, d] orientation via PE transpose
                ctxkd = p3a.tile([128, TQ, D], F16)
                for dt in range(DTL):
                    src = ctxT(b, dt)
                    for k2 in range(TQ):
                        pst = p3tr.tile([128, 128], F16)
                        nc.tensor.transpose(pst, src[:, k2 * 128:(k2 + 1) * 128],
                                            ident)
                        nc.vector.tensor_copy(
                            ctxkd[:, k2, dt * 128:(dt + 1) * 128], pst)
                # target^T = W_in^T.T @ ctx^T
                tgtT = p3a.tile([128, DTL, T], F16, name="tgtT")
                for mt in range(DTL):
                    ps = p3ps.tile([128, T], F32)
                    for kt in range(DTL):
                        nc.tensor.matmul(ps, lhsT=winT_sb[:, kt, mt, :],
                                         rhs=ctxT(b, kt),
                                         start=(kt == 0), stop=(kt == DTL - 1))
                    if mt % 2 == 0:
                        nc.scalar.copy(tgtT[:, mt, :], ps)
                    else:
                        nc.vector.tensor_copy(tgtT[:, mt, :], ps)
                # scores + softmax (no max-subtraction: scores are tiny)
                attn = p3b.tile([128, TQ, T], F16)
                rs = p3b.tile([128, TQ], F32, name="rs")
                rsr = p3b.tile([128, TQ], F32, name="rsr")
                exps = p3b.tile([128, TQ, T], F16, name="exps")
                for qt in range(TQ):
                    ps = p3ps.tile([128, T], F32)
                    for dt in range(DTL):
                        nc.tensor.matmul(ps,
                                         lhsT=tgtT[:, dt, qt * 128:(qt + 1) * 128],
                                         rhs=ctxT(b, dt),
                                         start=(dt == 0), stop=(dt == DTL - 1))
                    nc.scalar.activation(exps[:, qt, :], ps, AF.Exp,
                                         accum_out=rs[:, qt:qt + 1])
                    nc.vector.reciprocal(rsr[:, qt:qt + 1], rs[:, qt:qt + 1])
                    nc.vector.tensor_scalar_mul(attn[:, qt, :], exps[:, qt, :],
                                                rsr[:, qt:qt + 1])
                # attn^T
                attnT = p3b.tile([128, TQ, T], F16, name="attnT")
                for qt in range(TQ):
                    for k2 in range(TQ):
                        pst = p3tr.tile([128, 128], F16)
                        nc.tensor.transpose(
                            pst, attn[:, qt, k2 * 128:(k2 + 1) * 128], ident)
                        nc.vector.tensor_copy(
                            attnT[:, k2, qt * 128:(qt + 1) * 128], pst)
                # weighted^T = ctx_kd.T @ attn^T   [d, q]
                wtdT = p3a.tile([128, DTL, T], F16, name="wtdT")
                for mt in range(DTL):
                    ps = p3ps.tile([128, T], F32)
                    for k2 in range(TQ):
                        nc.tensor.matmul(ps,
                                         lhsT=ctxkd[:, k2, mt * 128:(mt + 1) * 128],
                                         rhs=attnT[:, k2, :],
                                         start=(k2 == 0), stop=(k2 == TQ - 1))
                    if mt % 2 == 0:
                        nc.scalar.copy(wtdT[:, mt, :], ps)
                    else:
                        nc.vector.tensor_copy(wtdT[:, mt, :], ps)
                # h_tilde^T = tanh(W_oa^T.T @ [wtd; ctx]) ; mean over q via accum
                scr = p3b.tile([128, T], F16, name="scr")
                for mt in range(DTL):
                    ps = p3ps.tile([128, T], F32)
                    for kt in range(2 * DTL):
                        rhs = wtdT[:, kt, :] if kt < DTL else ctxT(b, kt - DTL)
                        nc.tensor.matmul(ps, lhsT=woaT_sb[:, kt, mt, :], rhs=rhs,
                                         start=(kt == 0), stop=(kt == 2 * DTL - 1))
                    nc.scalar.activation(scr, ps, AF.Tanh,
                                         accum_out=ctxo[:, mt, b:b + 1])

            # ---- head (fp32) --------------------------------------------------
            for mt in range(2):
                psy = p3tr.tile([128, BA], F32, name="psy")
                for kt in range(DTL):
                    nc.tensor.matmul(psy, lhsT=woutT_sb[:, kt, mt, :],
                                     rhs=ctxo[:, kt, :],
                                     start=(kt == 0), stop=(kt == DTL - 1))
                ysb = p3b.tile([128, BA], F32, name="ysb")
                nc.scalar.activation(ysb, psy, AF.Tanh,
                                     bias=bout_sb[:, mt:mt + 1])
                nc.gpsimd.dma_start(out=yT[mt * 128:(mt + 1) * 128, :], in_=ysb)

        singles.release()
    nc.finalize()
    return nc


# -------------------------------------------------------------------------
# host side
# -------------------------------------------------------------------------

_NC_CACHE = {}


def _get_nc(T=256):
    if T not in _NC_CACHE:
        _NC_CACHE[T] = build(T)
    return _NC_CACHE[T]


def _perm_gates(w):
    """Reorder pytorch gate blocks (i,f,g,o) -> device (g,i,f,o) along axis 0."""
    blocks = np.split(w, 4, axis=0)
    return np.concatenate([blocks[k] for k in GATE_PERM], axis=0)


def make_in_maps(inputs, T=256):
    f16 = np.float16
    emb = np.asarray(inputs["embed_table"], np.float32)
    ids = np.asarray(inputs["inputs"], np.int64)
    x_all = emb[ids]  # [B, T, E]

    pD = np.concatenate([np.arange(H, D), np.arange(0, H)])  # swap halves of D
    p2D = np.concatenate([pD, pD + D])

    in_maps = []
    for c in range(NCORES):
        p = c % 4
        fwd = c < 4
        seqs = np.arange(16 * p, 16 * p + 16)
        if not fwd:
            seqs = np.concatenate([seqs[8:], seqs[:8]])
        xc = x_all[seqs][:, :T, :]  # [16, T, E]
        if not fwd:
            xc = xc[:, ::-1, :]
        xT = np.zeros((EP, T * BL), f16)
        xT[:E] = np.ascontiguousarray(xc.transpose(2, 1, 0)).reshape(E, T * BL)
        xT[E] = 1.0

        sfx = "f" if fwd else "b"
        w_ih = _perm_gates(np.asarray(inputs[f"w_ih_{sfx}"], np.float32))
        w_hh = _perm_gates(np.asarray(inputs[f"w_hh_{sfx}"], np.float32))
        b_sum = _perm_gates((np.asarray(inputs[f"b_ih_{sfx}"], np.float32)
                             + np.asarray(inputs[f"b_hh_{sfx}"], np.float32))[:, None])[:, 0]
        wihT = np.zeros((EP, G), f16)
        wihT[:E] = w_ih.T
        wihT[E] = b_sum
        whhT = w_hh.T.astype(f16)

        w_in = np.asarray(inputs["w_in"], np.float32)
        w_oa = np.asarray(inputs["w_out_attn"], np.float32)
        if not fwd:
            w_in = w_in[pD][:, pD]
            w_oa = w_oa[:, p2D]
        winT = w_in.T.astype(f16)
        woaT = w_oa.T.astype(f16)

        woutT = (np.asarray(inputs["w_out"], np.float32).T / T).astype(np.float32)
        boutc = np.asarray(inputs["b_out"], np.float32)[:, None]

        in_maps.append({
            "xT": xT, "wihT": wihT, "whhT": whhT,
            "winT": winT, "woaT": woaT,
            "woutT": woutT, "bout": boutc,
            "peerblk": np.array([[(c + 4) % NCORES]], np.uint32),
        })
    return in_maps


def assemble_output(results, inputs, T=256):
    y = np.zeros((B, OUT), np.float32)
    for c in range(NCORES):
        p = c % 4
        att = np.arange(16 * p, 16 * p + 8) if c < 4 else \
            np.arange(16 * p + 8, 16 * p + 16)
        y[att] = results[c]["yT"].T
    yd = y.astype(np.float64)
    mu = yd.mean(0)
    var = ((yd - mu) ** 2).mean(0)
    gamma = np.asarray(inputs["gamma"], np.float64)
    beta = np.asarray(inputs["beta"], np.float64)
    out = gamma * (yd - mu) / np.sqrt(var + EPS) + beta
    return out.astype(np.float32)


def kernel(**inputs) -> np.ndarray:
    T = np.asarray(inputs["inputs"]).shape[1]
    nc = _get_nc(T)
    in_maps = make_in_maps(inputs, T)
    res = run_bass_kernel_spmd(nc, in_maps, core_ids=list(range(NCORES)))
    return assemble_output(res.results, inputs, T)


# revision 11
# speedup vs baseline: 1.1080x; 1.1080x over previous
"""Trainium2 Bass kernel for nn_AttnNet (BiLSTM + SoftDotAttention + head + BatchNorm).

Strategy (8 NeuronCores, direction-split data parallel):
  - Pair p = (core p, core p+4) jointly owns 16 of the 64 sequences.
    Core p runs the FORWARD LSTM for those 16 seqs; core p+4 runs the
    BACKWARD LSTM (fed time-reversed embeddings, so the on-device
    recurrence is identical SPMD code).
  - Embedding gather happens on host (memory-bound; inputs are shipped
    pre-gathered and pre-transposed per core).
  - Phase 1 (device): xpre = [W_ih^T; bias]^T @ [x; 1]  for all t,b.
  - Phase 2 (device): 256 sequential LSTM steps; per step a
    [512x2048]^T weight-stationary matmul over the 16-seq batch.
    Gates are processed in two h-halves so the ACT/DVE nonlinearity
    tail of one half overlaps the tensor engine work of the other and
    of the next step.
  - AllGather exchanges the 8 "for the pair" hidden histories; each
    core then holds ctx = [h_local, h_remote] for its 8 attended seqs.
    The remote half is time-reversed once in SBUF (negative-stride DVE
    read); this is mathematically safe: attention + mean over T is
    permutation invariant in t, and W_in/W_oa are half-swapped on host
    for the backward cores.
  - Phase 3 (device): SoftDotAttention per seq in fp16 (tensor engine),
    output head in fp32, tanh; y^T [256, 8] out per core.
  - BatchNorm (training-mode batch stats over the full 64-batch) on host.

All matmuls run in float16 (full PE rate, 8x finer mantissa than bf16 --
the BatchNorm at the end amplifies batch-varying errors ~30x, bf16 fails).

Gate-tile layout on device (16 tiles of 128 gate rows):
  bi = half*8 + gtype*2 + htw, gtype in (g,i,f,o), h-tile ht = half*2+htw.
  So per half: g = [8h,8h+2), i,f,o = [8h+2,8h+8) (one tanh + one sigmoid).
"""

import os
import numpy as np
import ml_dtypes  # noqa: F401

import concourse.bass as bass
import concourse.tile as tile
from concourse import bacc
from concourse import mybir
from concourse.bass_utils import run_bass_kernel_spmd
from concourse.masks import make_identity

F16 = mybir.dt.float16
F32 = mybir.dt.float32
U32 = mybir.dt.uint32
AF = mybir.ActivationFunctionType
ALU = mybir.AluOpType

B, E, H, OUT = 64, 300, 512, 256
D = 2 * H
NCORES = 8
BL = 16        # sequences per pair (local batch in the LSTM)
BA = 8         # sequences attended per core
HT = H // 128  # 4 h-tiles
G = 4 * H      # 2048 gate rows
GT = G // 128  # 16 gate tiles
EP = 384       # padded E + bias row (3 k-tiles)
KT1 = EP // 128
DTL = D // 128  # 8 d-tiles
EPS = 1e-5

# pytorch gate block (i,f,g,o) index for device gtype order (g,i,f,o)
PT_OF_GTYPE = [2, 0, 1, 3]


def _gate_row_perm():
    """Device gate-row permutation: device row r <- pytorch row perm[r]."""
    perm = np.zeros(G, np.int64)
    for bi in range(GT):
        half, rem = bi // 8, bi % 8
        gtype, htw = rem // 2, rem % 2
        pt = PT_OF_GTYPE[gtype]
        ht = half * 2 + htw
        perm[bi * 128:(bi + 1) * 128] = pt * H + ht * 128 + np.arange(128)
    return perm


GATE_ROW_PERM = _gate_row_perm()


def _rev_t(ap, tdim_idx):
    """Return ap with dim `tdim_idx` (index into ap.ap, partition dim at 0)
    reversed (negative stride)."""
    dims = [list(d) for d in ap.ap]
    stride, size = dims[tdim_idx]
    new_off = ap.offset + (size - 1) * stride
    dims[tdim_idx] = [-stride, size]
    return bass.AP(tensor=ap.tensor, offset=new_off, ap=dims)


def _pair_gc(act_cur, h):
    """AP over [g-half, c-half] of the act tile: dims [128, 2, 2, BL]."""
    a = act_cur[:, 8 * h:8 * h + 2, :]
    dims = [list(x) for x in a.ap]
    # dims: [[pstride,128],[BL,2],[1,BL]] ; insert pair dim (g -> c)
    pair_stride = (16 + 2 * h - 8 * h) * BL
    new = [dims[0], [pair_stride, 2], dims[1], dims[2]]
    return bass.AP(tensor=a.tensor, offset=a.offset, ap=new)


def build(T=256):
    nc = bacc.Bacc('TRN2', target_bir_lowering=False, debug=False,
                   num_devices=NCORES)
    NTOK = T * BL
    TQ = T // 128  # q/k tiles in attention

    xT = nc.declare_dram_parameter("xT", [EP, NTOK], F16, False)
    wihT = nc.declare_dram_parameter("wihT", [EP, G], F16, False)
    whhT = nc.declare_dram_parameter("whhT", [H, G], F16, False)
    winT = nc.declare_dram_parameter("winT", [D, D], F16, False)
    woaT = nc.declare_dram_parameter("woaT", [2 * D, D], F16, False)
    woutT = nc.declare_dram_parameter("woutT", [D, OUT], F32, False)
    bout = nc.declare_dram_parameter("bout", [OUT, 1], F32, False)
    peer = nc.declare_dram_parameter("peerblk", [1, 1], U32, False)
    yT = nc.declare_dram_parameter("yT", [OUT, BA], F32, True)

    xpre_d = nc.dram_tensor("xpre_d", [GT, 128, T, BL], F16)
    bnc_in = nc.dram_tensor("bnc_in", [128, BA, T * HT], F16)
    bnc_out = nc.dram_tensor("bnc_out", [NCORES, 128, BA, T * HT], F16,
                             addr_space="Shared")

    with tile.TileContext(nc) as tc:
        singles = tc.alloc_tile_pool(name="singles", bufs=1)

        # ---- resident weights -------------------------------------------------
        wihT_sb = singles.tile([128, KT1, GT, 128], F16)
        nc.gpsimd.dma_start(out=wihT_sb,
                          in_=wihT.rearrange("(kt p) (gt c) -> p kt gt c",
                                             p=128, c=128))
        whhT_sb = singles.tile([128, HT, GT, 128], F16)
        nc.gpsimd.dma_start(out=whhT_sb,
                          in_=whhT.rearrange("(ht p) (gt c) -> p ht gt c",
                                             p=128, c=128))
        winT_sb = singles.tile([128, DTL, DTL, 128], F16)
        nc.gpsimd.dma_start(out=winT_sb,
                          in_=winT.rearrange("(kt p) (mt c) -> p kt mt c",
                                             p=128, c=128))
        woaT_sb = singles.tile([128, 2 * DTL, DTL, 128], F16)
        nc.gpsimd.dma_start(out=woaT_sb,
                          in_=woaT.rearrange("(kt p) (mt c) -> p kt mt c",
                                             p=128, c=128))
        woutT_sb = singles.tile([128, DTL, 2, 128], F32)
        nc.gpsimd.dma_start(out=woutT_sb,
                          in_=woutT.rearrange("(kt p) (mt c) -> p kt mt c",
                                              p=128, c=128))
        bout_sb = singles.tile([128, 2], F32)
        nc.gpsimd.dma_start(out=bout_sb,
                          in_=bout.rearrange("(mt p) one -> p (mt one)", p=128))
        ident = singles.tile([128, 128], F16)
        make_identity(nc, ident)

        h_hist = singles.tile([128, BL, T + 1, HT], F16)
        nc.vector.memset(h_hist[:, :, 0, :], 0.0)
        # act ping-pong: slots [0:16] gates (bi order), [16:20] c-state (ht order)
        act_pp = [singles.tile([128, 20, BL], F32, name=f"act{i}")
                  for i in range(2)]
        nc.vector.memset(act_pp[0][:, 16:20, :], 0.0)

        # ---- phase 1: xpre ----------------------------------------------------
        xT_v = xT.rearrange("(kt p) n -> p kt n", p=128)
        NSL = NTOK // 512
        with tc.tile_pool(name="p1x", bufs=3) as p1x, \
             tc.tile_pool(name="p1ps", bufs=2, space="PSUM") as p1ps, \
             tc.tile_pool(name="p1st", bufs=4) as p1st:
            for s in range(NSL):
                xt_t = p1x.tile([128, KT1, 512], F16)
                nc.gpsimd.dma_start(out=xt_t, in_=xT_v[:, :, s * 512:(s + 1) * 512])
                for gt in range(GT):
                    ps = p1ps.tile([128, 512], F32)
                    for kt in range(KT1):
                        nc.tensor.matmul(ps, lhsT=wihT_sb[:, kt, gt, :],
                                         rhs=xt_t[:, kt, :],
                                         start=(kt == 0), stop=(kt == KT1 - 1))
                    st = p1st.tile([128, 512], F16)
                    if gt % 2 == 0:
                        nc.scalar.copy(st, ps)
                    else:
                        nc.vector.tensor_copy(st, ps)
                    nc.gpsimd.dma_start(
                        out=xpre_d[gt, :, s * (512 // BL):(s + 1) * (512 // BL), :],
                        in_=st.rearrange("p (t b) -> p t b", b=BL))

        # ---- phase 2: recurrence ---------------------------------------------
        xpre_v = xpre_d.rearrange("gt p t b -> p gt t b")
        CH = 32  # t-chunk per xpre DMA
        with tc.tile_pool(name="p2x", bufs=2) as p2x, \
             tc.tile_pool(name="p2ps", bufs=2, space="PSUM") as p2ps, \
             tc.tile_pool(name="p2s", bufs=8) as p2s:
            xpc = None
            for t in range(T):
                tl = t % CH
                if tl == 0:
                    xpc = p2x.tile([128, GT, CH, BL], F16)
                    nc.gpsimd.dma_start(out=xpc, in_=xpre_v[:, :, t:t + CH, :])
                act_cur, act_nxt = act_pp[t % 2], act_pp[(t + 1) % 2]
                # psum per half: separate banks so the first half's tail can
                # start while the PE still writes the second half
                ps_h = [p2ps.tile([128, 8, BL], F32, name=f"ps2{i}")
                        for i in range(2)]
                for i in range(2):
                    nc.vector.tensor_copy(ps_h[i], xpc[:, 8 * i:8 * i + 8, tl, :])
                for ht in range(HT):
                    rhs = h_hist[:, :, t, ht]
                    for bi in range(GT):
                        nc.tensor.matmul(
                            ps_h[bi // 8][:, bi % 8, :],
                            lhsT=whhT_sb[:, ht, bi, :], rhs=rhs,
                            start=False, stop=(ht == HT - 1),
                            skip_group_check=True)
                for h in range(2):
                    ps = ps_h[h]
                    base = 8 * h
                    nc.scalar.activation(act_cur[:, base:base + 2, :],
                                         ps[:, 0:2, :], AF.Tanh)
                    nc.scalar.activation(act_cur[:, base + 2:base + 8, :],
                                         ps[:, 2:8, :], AF.Sigmoid)
                    tmp = p2s.tile([128, 2, 2, BL], F32, name="tmp")
                    nc.vector.tensor_tensor(
                        tmp,
                        act_cur[:, base + 2:base + 6, :].rearrange(
                            "p (pr w) b -> p pr w b", pr=2),
                        _pair_gc(act_cur, h), ALU.mult)
                    cs = act_nxt[:, 16 + 2 * h:18 + 2 * h, :]
                    nc.vector.tensor_tensor(cs, tmp[:, 0], tmp[:, 1], ALU.add)
                    thc = p2s.tile([128, 2, BL], F32, name="thc")
                    nc.scalar.activation(thc, cs, AF.Tanh)
                    h_out = h_hist[:, :, t + 1, 2 * h:2 * h + 2].rearrange(
                        "p b w -> p w b")
                    nc.vector.tensor_tensor(h_out, act_cur[:, base + 6:base + 8, :],
                                            thc, ALU.mult)

        # ---- exchange ---------------------------------------------------------
        nc.gpsimd.dma_start(
            out=bnc_in.rearrange("p b (t h) -> p b t h", h=HT),
            in_=h_hist[:, BA:BL, 1:T + 1, :])
        nc.gpsimd.collective_compute(
            "AllGather", ALU.bypass,
            ins=[bnc_in[:]], outs=[bnc_out[:]],
            replica_groups=[list(range(NCORES))])

        h_rem = singles.tile([128, BA, T, HT], F16)
        peer_sb = singles.tile([1, 1], U32)
        nc.gpsimd.dma_start(out=peer_sb, in_=peer[:, :])
        reg = nc.gpsimd.alloc_register("peerblk_reg")
        nc.gpsimd.reg_load(reg, peer_sb[0:1, 0:1])
        sv = nc.gpsimd.snap(reg, donate=True, min_val=0, max_val=NCORES - 1)
        nc.gpsimd.dma_start(
            out=h_rem,
            in_=bnc_out.rearrange("r p b (t h) -> r p b t h", h=HT)[
                bass.ds(sv, 1), :, :, :, :].rearrange(
                    "o p b t h -> p (o b) t h"))
        # time-reverse the peer block in SBUF (negative-stride DVE read)
        h_rev = singles.tile([128, BA, T, HT], F16)
        nc.vector.tensor_copy(h_rev, _rev_t(h_rem, 2))

        def ctxT(b, dt):
            if dt < HT:
                return h_hist[:, b, 1:T + 1, dt]
            return h_rev[:, b, :, dt - HT]

        # ---- phase 3: attention ----------------------------------------------
        ctxo = singles.tile([128, DTL, BA], F32)
        with tc.tile_pool(name="p3ps", bufs=4, space="PSUM") as p3ps, \
             tc.tile_pool(name="p3tr", bufs=2, space="PSUM") as p3tr, \
             tc.tile_pool(name="p3a", bufs=2) as p3a, \
             tc.tile_pool(name="p3b", bufs=2) as p3b:
            for b in range(BA):
                # ctx in [k, d] orientation via PE transpose
                ctxkd = p3a.tile([128, TQ, D], F16)
                for dt in range(DTL):
                    src = ctxT(b, dt)
                    for k2 in range(TQ):
                        pst = p3tr.tile([128, 128], F16)
                        nc.tensor.transpose(pst, src[:, k2 * 128:(k2 + 1) * 128],
                                            ident)
                        if (dt + k2) % 2 == 0:
                            nc.vector.tensor_copy(
                                ctxkd[:, k2, dt * 128:(dt + 1) * 128], pst)
                        else:
                            nc.scalar.copy(
                                ctxkd[:, k2, dt * 128:(dt + 1) * 128], pst)
                # target^T = W_in^T.T @ ctx^T
                tgtT = p3a.tile([128, DTL, T], F16, name="tgtT")
                for mt in range(DTL):
                    ps = p3ps.tile([128, T], F32)
                    for kt in range(DTL):
                        nc.tensor.matmul(ps, lhsT=winT_sb[:, kt, mt, :],
                                         rhs=ctxT(b, kt),
                                         start=(kt == 0), stop=(kt == DTL - 1))
                    if mt % 2 == 0:
                        nc.scalar.copy(tgtT[:, mt, :], ps)
                    else:
                        nc.vector.tensor_copy(tgtT[:, mt, :], ps)
                # scores + softmax (no max-subtraction: scores are tiny)
                attn = p3b.tile([128, TQ, T], F16)
                rs = p3b.tile([128, TQ], F32, name="rs")
                rsr = p3b.tile([128, TQ], F32, name="rsr")
                exps = p3b.tile([128, TQ, T], F16, name="exps")
                for qt in range(TQ):
                    ps = p3ps.tile([128, T], F32)
                    for dt in range(DTL):
                        nc.tensor.matmul(ps,
                                         lhsT=tgtT[:, dt, qt * 128:(qt + 1) * 128],
                                         rhs=ctxT(b, dt),
                                         start=(dt == 0), stop=(dt == DTL - 1))
                    nc.scalar.activation(exps[:, qt, :], ps, AF.Exp,
                                         accum_out=rs[:, qt:qt + 1])
                    nc.vector.reciprocal(rsr[:, qt:qt + 1], rs[:, qt:qt + 1])
                    nc.vector.tensor_scalar_mul(attn[:, qt, :], exps[:, qt, :],
                                                rsr[:, qt:qt + 1])
                # attn^T
                attnT = p3b.tile([128, TQ, T], F16, name="attnT")
                for qt in range(TQ):
                    for k2 in range(TQ):
                        pst = p3tr.tile([128, 128], F16)
                        nc.tensor.transpose(
                            pst, attn[:, qt, k2 * 128:(k2 + 1) * 128], ident)
                        if (qt + k2) % 2 == 0:
                            nc.vector.tensor_copy(
                                attnT[:, k2, qt * 128:(qt + 1) * 128], pst)
                        else:
                            nc.scalar.copy(
                                attnT[:, k2, qt * 128:(qt + 1) * 128], pst)
                # weighted^T = ctx_kd.T @ attn^T   [d, q]
                wtdT = p3a.tile([128, DTL, T], F16, name="wtdT")
                for mt in range(DTL):
                    ps = p3ps.tile([128, T], F32)
                    for k2 in range(TQ):
                        nc.tensor.matmul(ps,
                                         lhsT=ctxkd[:, k2, mt * 128:(mt + 1) * 128],
                                         rhs=attnT[:, k2, :],
                                         start=(k2 == 0), stop=(k2 == TQ - 1))
                    if mt % 2 == 0:
                        nc.scalar.copy(wtdT[:, mt, :], ps)
                    else:
                        nc.vector.tensor_copy(wtdT[:, mt, :], ps)
                # h_tilde^T = tanh(W_oa^T.T @ [wtd; ctx]); mean over q via accum
                scr = p3b.tile([128, T], F16, name="scr")
                for mt in range(DTL):
                    ps = p3ps.tile([128, T], F32)
                    for kt in range(2 * DTL):
                        rhs = wtdT[:, kt, :] if kt < DTL else ctxT(b, kt - DTL)
                        nc.tensor.matmul(ps, lhsT=woaT_sb[:, kt, mt, :], rhs=rhs,
                                         start=(kt == 0), stop=(kt == 2 * DTL - 1))
                    nc.scalar.activation(scr, ps, AF.Tanh,
                                         accum_out=ctxo[:, mt, b:b + 1])

            # ---- head (fp32) --------------------------------------------------
            for mt in range(2):
                psy = p3tr.tile([128, BA], F32, name="psy")
                for kt in range(DTL):
                    nc.tensor.matmul(psy, lhsT=woutT_sb[:, kt, mt, :],
                                     rhs=ctxo[:, kt, :],
                                     start=(kt == 0), stop=(kt == DTL - 1))
                ysb = p3b.tile([128, BA], F32, name="ysb")
                nc.scalar.activation(ysb, psy, AF.Tanh,
                                     bias=bout_sb[:, mt:mt + 1])
                nc.gpsimd.dma_start(out=yT[mt * 128:(mt + 1) * 128, :], in_=ysb)

        singles.release()
    nc.finalize()
    return nc


# -------------------------------------------------------------------------
# host side
# -------------------------------------------------------------------------

_NC_CACHE = {}


def _get_nc(T=256):
    if T not in _NC_CACHE:
        _NC_CACHE[T] = build(T)
    return _NC_CACHE[T]


def make_in_maps(inputs, T=256):
    f16 = np.float16
    emb = np.asarray(inputs["embed_table"], np.float32)
    ids = np.asarray(inputs["inputs"], np.int64)
    x_all = emb[ids]  # [B, T, E]

    pD = np.concatenate([np.arange(H, D), np.arange(0, H)])  # swap halves of D
    p2D = np.concatenate([pD, pD + D])

    in_maps = []
    for c in range(NCORES):
        p = c % 4
        fwd = c < 4
        seqs = np.arange(16 * p, 16 * p + 16)
        if not fwd:
            seqs = np.concatenate([seqs[8:], seqs[:8]])
        xc = x_all[seqs][:, :T, :]  # [16, T, E]
        if not fwd:
            xc = xc[:, ::-1, :]
        xT = np.zeros((EP, T * BL), f16)
        xT[:E] = np.ascontiguousarray(xc.transpose(2, 1, 0)).reshape(E, T * BL)
        xT[E] = 1.0

        sfx = "f" if fwd else "b"
        w_ih = np.asarray(inputs[f"w_ih_{sfx}"], np.float32)[GATE_ROW_PERM]
        w_hh = np.asarray(inputs[f"w_hh_{sfx}"], np.float32)[GATE_ROW_PERM]
        b_sum = (np.asarray(inputs[f"b_ih_{sfx}"], np.float32)
                 + np.asarray(inputs[f"b_hh_{sfx}"], np.float32))[GATE_ROW_PERM]
        wihT = np.zeros((EP, G), f16)
        wihT[:E] = w_ih.T
        wihT[E] = b_sum
        whhT = w_hh.T.astype(f16)

        w_in = np.asarray(inputs["w_in"], np.float32)
        w_oa = np.asarray(inputs["w_out_attn"], np.float32)
        if not fwd:
            w_in = w_in[pD][:, pD]
            w_oa = w_oa[:, p2D]
        winT = w_in.T.astype(f16)
        woaT = w_oa.T.astype(f16)

        woutT = (np.asarray(inputs["w_out"], np.float32).T / T).astype(np.float32)
        boutc = np.asarray(inputs["b_out"], np.float32)[:, None]

        in_maps.append({
            "xT": xT, "wihT": wihT, "whhT": whhT,
            "winT": winT, "woaT": woaT,
            "woutT": woutT, "bout": boutc,
            "peerblk": np.array([[(c + 4) % NCORES]], np.uint32),
        })
    return in_maps


def assemble_output(results, inputs, T=256):
    y = np.zeros((B, OUT), np.float32)
    for c in range(NCORES):
        p = c % 4
        att = np.arange(16 * p, 16 * p + 8) if c < 4 else \
            np.arange(16 * p + 8, 16 * p + 16)
        y[att] = results[c]["yT"].T
    yd = y.astype(np.float64)
    mu = yd.mean(0)
    var = ((yd - mu) ** 2).mean(0)
    gamma = np.asarray(inputs["gamma"], np.float64)
    beta = np.asarray(inputs["beta"], np.float64)
    out = gamma * (yd - mu) / np.sqrt(var + EPS) + beta
    return out.astype(np.float32)


def kernel(**inputs) -> np.ndarray:
    T = np.asarray(inputs["inputs"]).shape[1]
    nc = _get_nc(T)
    in_maps = make_in_maps(inputs, T)
    res = run_bass_kernel_spmd(nc, in_maps, core_ids=list(range(NCORES)))
    return assemble_output(res.results, inputs, T)
